# revision 24
# baseline (speedup 1.0000x reference)
"""Trainium2 Bass kernel for a dense transformer block (QKV+gate proj, RoPE,
QK-RMSNorm, causal SDPA, output-RMSNorm + SiLU gate, output projection).

Sharding: tensor-parallel over heads across 8 NeuronCores (2 heads/core).
Wq/Wk/Wv/Wg split column-wise, attention fully local per core; the per-core
attention outputs are exchanged with an AllToAll (token-sharding), after
which each core computes the FULL output projection for its 256-token slice
(per batch) with the full Wo resident in SBUF.  This replaces the 8x-larger
AllGather wire volume of the row/column-parallel variants.

Matmuls run in bf16.  RoPE uses a sign-folded sin table (rows 0-63 hold
-sin, 64-127 hold +sin) so the rotate-half combine is a single add.
The softmax denominator is never computed: RMSNorm(y/s) == RMSNorm(y) up to
the (negligible) eps term, so only sum(y^2) is reduced via a ones-matmul.
"""

import os
import sys

for _p in ("/opt/trn_rl_repo", "/root/.axon_site/_ro/trn_rl_repo"):
    if os.path.isdir(_p) and _p not in sys.path:
        sys.path.insert(0, _p)

import numpy as np

import concourse.bass as bass
import concourse.mybir as mybir
from concourse import bacc
from concourse.bass_utils import run_bass_kernel_spmd
from concourse.tile import TileContext

B, T, HID = 2, 2048, 2048
H, D = 16, 128
NCORES = 8
HC = H // NCORES          # heads per core = 2
DC = HC * D               # 256 head-dims per core
TC = T // NCORES          # 256 tokens per core (final projection shard)
BT = B * T                # 4096 tokens
KT = HID // 128           # 16 contraction tiles
EPS = 1e-5
SCALE = 1.0 / float(np.sqrt(D))
NEG = -3.0e38

F32 = mybir.dt.float32
BF16 = mybir.dt.bfloat16
AF = mybir.ActivationFunctionType
ALU = mybir.AluOpType

MMDT = BF16

LAST_EXEC_TIME_NS = None
LAST_RESULT = None
_CACHED_NC = None


def _proj_sweep(nc, tc, xT_r, w_aps, post):
    """One sweep over xT computing 2 matrices (4 head-groups) in transposed
    layout: psum[dhead 128, tok 512] double-buffered, accumulated over 16
    k-tiles."""
    with tc.tile_pool(name="sweep_w", bufs=1) as wpool, \
         tc.tile_pool(name="sweep_x", bufs=3) as xpool, \
         tc.tile_pool(name="sweep_ps", bufs=2, space="PSUM") as pps, \
         tc.tile_pool(name="sweep_t", bufs=2) as tpool:
        wsb = []
        for mi, w_ap in enumerate(w_aps):
            w_t = wpool.tile([128, KT, DC], MMDT, tag=f"w{mi}", name=f"w{mi}")
            nc.sync.dma_start(out=w_t, in_=w_ap)
            wsb.append(w_t)
        pending = None
        for nb in range(BT // 512):
            nbb, half = nb // 2, nb % 2
            ps = {}
            for mi in range(2):
                for m in range(HC):
                    ps[(mi, m)] = pps.tile([128, 512], F32,
                                           tag=f"pp{mi}{m}", name=f"pp{mi}{m}")
            for k in range(KT):
                xk = xpool.tile([128, 512], MMDT, tag=f"xk{half}", name="xk",
                                bufs=6)
                nc.sync.dma_start(
                    out=xk, in_=xT_r[k, nbb, :, half * 512:(half + 1) * 512])
                for mi in range(2):
                    for m in range(HC):
                        nc.tensor.matmul(
                            ps[(mi, m)], wsb[mi][:, k, m * 128:(m + 1) * 128],
                            xk, start=(k == 0), stop=(k == KT - 1))
            # defer evacuation by one token-block: keeps the posts' PE work
            # (row-sum matmuls / transposes) from head-of-line blocking the
            # next block's main matmuls while ACT catches up
            if pending is not None:
                nbp, psp = pending
                for mi in range(2):
                    for m in range(HC):
                        post[mi](psp[(mi, m)], m, nbp, tpool, pps,
                                 f"pp{mi}{m}")
            pending = (nb, ps)
        nbp, psp = pending
        for mi in range(2):
            for m in range(HC):
                post[mi](psp[(mi, m)], m, nbp, tpool, pps, f"pp{mi}{m}")


def _build_nc():
    nc = bacc.Bacc("TRN2", target_bir_lowering=False, debug=False,
                   num_devices=NCORES)

    xT = nc.dram_tensor("xT", [KT, BT // 1024, 128, 1024], MMDT,
                        kind="ExternalInput").ap()
    wq = nc.dram_tensor("wq", [128, KT, DC], MMDT, kind="ExternalInput").ap()
    wk = nc.dram_tensor("wk", [128, KT, DC], MMDT, kind="ExternalInput").ap()
    wv = nc.dram_tensor("wv", [128, KT, DC], MMDT, kind="ExternalInput").ap()
    wg = nc.dram_tensor("wg", [128, KT, DC], MMDT, kind="ExternalInput").ap()
    # per-core column slice of the output projection (o_norm folded in)
    wo = nc.dram_tensor("wo", [128, KT, DC], MMDT, kind="ExternalInput").ap()
    cos2 = nc.dram_tensor("cos2", [128, BT], F32, kind="ExternalInput").ap()
    # sign-folded: rows 0-63 = -sin, rows 64-127 = +sin
    sin2 = nc.dram_tensor("sin2", [128, BT], F32, kind="ExternalInput").ap()
    # causal 0/1 mask for diagonal tiles (k<=q -> 1)
    mask01 = nc.dram_tensor("mask01", [128, 128], MMDT, kind="ExternalInput").ap()
    ones_in = nc.dram_tensor("ones_in", [128, 128], MMDT, kind="ExternalInput").ap()
    ident_in = nc.dram_tensor("ident_in", [128, 128], F32, kind="ExternalInput").ap()
    qrw = nc.dram_tensor("qrw", [128, 1], F32, kind="ExternalInput").ap()
    krw = nc.dram_tensor("krw", [128, 1], F32, kind="ExternalInput").ap()

    outT = nc.dram_tensor("outT", [DC, BT], F32, kind="ExternalOutput").ap()
    sgd = nc.dram_tensor("sgd", [DC, BT], MMDT).ap()       # silu(gate) spill
    TH = T // 2          # AllGather granularity: half a batch of tokens
    ag_in = [[nc.dram_tensor(f"ag_in{b}_{h}", [DC, TH], MMDT).ap()
              for h in range(2)] for b in range(B)]
    yall = [[nc.dram_tensor(f"yall{b}_{h}", [NCORES * DC, TH], MMDT,
                            addr_space="Shared").ap() for h in range(2)]
            for b in range(B)]

    with TileContext(nc) as tc:
        with tc.tile_pool(name="const", bufs=1) as const:
            ones_r = const.tile([128, 128], MMDT)
            nc.sync.dma_start(out=ones_r, in_=ones_in)
            epsb = const.tile([128, 1], F32)
            nc.vector.memset(epsb, EPS)
            zerob = const.tile([128, 1], F32)
            nc.vector.memset(zerob, 0.0)

            with tc.tile_pool(name="persist", bufs=1) as persist:
                # Wo column slice in SBUF; prefetch on the gpsimd DMA queue
                wo_sb = persist.tile([128, KT, DC], MMDT, tag="wo", name="wo")
                nc.gpsimd.dma_start(out=wo_sb, in_=wo)

                # final (rope+rms applied) qT/kT per head, bf16 [d, b*t]
                qTf = [persist.tile([128, BT], MMDT, tag=f"qTf{m}",
                                    name=f"qTf{m}") for m in range(HC)]
                kTf = [persist.tile([128, BT], MMDT, tag=f"kTf{m}",
                                    name=f"kTf{m}") for m in range(HC)]

                # ---- sweep A: q, k (rope + rms fused into evacuation) ----
                if True:
                    # persist (not a scoped pool) so closing doesn't stall
                    # the sweep-B pools on the rope evacuation drain.
                    # off the sync queue so w/x tile loads start immediately
                    cos_sb = persist.tile([128, BT], F32, tag="cos")
                    nc.scalar.dma_start(out=cos_sb, in_=cos2)
                    sin_sb = persist.tile([128, BT], F32, tag="sin")
                    nc.scalar.dma_start(out=sin_sb, in_=sin2)
                    qrw_sb = persist.tile([128, 1], F32, tag="qrw")
                    nc.scalar.dma_start(out=qrw_sb, in_=qrw)
                    krw_sb = persist.tile([128, 1], F32, tag="krw")
                    nc.scalar.dma_start(out=krw_sb, in_=krw)

                    def make_qk_post(dest, w_scalar):
                        def post(ps, m, nb, tpool, pps, pstag):
                            c0, c1 = nb * 512, (nb + 1) * 512
                            stage = tpool.tile([128, 512], F32, tag="stage",
                                               name="stage")
                            nc.scalar.copy(stage, ps)
                            sw = tpool.tile([128, 512], F32, tag="sw",
                                            name="sw")
                            nc.scalar.dma_start(out=sw[0:64, :],
                                                in_=stage[64:128, :])
                            nc.scalar.dma_start(out=sw[64:128, :],
                                                in_=stage[0:64, :])
                            sqq = tpool.tile([128, 512], MMDT, tag="sq",
                                             name="sq")
                            nc.scalar.activation(out=sqq, in_=ps,
                                                 func=AF.Square)
                            ss = pps.tile([128, 512], F32, tag=pstag,
                                          name="ss")
                            nc.tensor.matmul(ss, ones_r, sqq,
                                             start=True, stop=True)
                            fac = tpool.tile([128, 512], F32, tag="fac",
                                             name="fac")
                            nc.scalar.activation(out=fac, in_=ss,
                                                 func=AF.Abs_reciprocal_sqrt,
                                                 scale=1.0 / float(D),
                                                 bias=epsb)
                            u = tpool.tile([128, 512], F32, tag="u", name="u")
                            nc.vector.tensor_mul(u, stage, cos_sb[:, c0:c1])
                            w = tpool.tile([128, 512], F32, tag="w", name="w")
                            nc.gpsimd.tensor_mul(w, sw, sin_sb[:, c0:c1])
                            ro = tpool.tile([128, 512], F32, tag="ro",
                                            name="ro")
                            nc.vector.tensor_add(ro, u, w)
                            # dest = (ro * w[d]) * factor  (fused)
                            nc.vector.scalar_tensor_tensor(
                                out=dest[m][:, c0:c1], in0=ro, scalar=w_scalar,
                                in1=fac, op0=ALU.mult, op1=ALU.mult)
                        return post

                    _proj_sweep(nc, tc, xT, [wq, wk],
                                [make_qk_post(qTf, qrw_sb),
                                 make_qk_post(kTf, krw_sb)])

                # ---- sweep B: v (transpose to [tok, d]), silu(g) spill ----
                v_sb = [persist.tile([128, BT // 128, 128], MMDT, tag=f"v{m}",
                                     name=f"v{m}") for m in range(HC)]
                with tc.tile_pool(name="identp", bufs=1) as ip:
                    ident = ip.tile([128, 128], F32)
                    nc.sync.dma_start(out=ident, in_=ident_in)

                    def v_post(ps, m, nb, tpool, pps, pstag):
                        stage = tpool.tile([128, 512], F32, tag="vstage",
                                           name="stage")
                        nc.scalar.copy(stage, ps)
                        for j in range(4):
                            tp = pps.tile([128, 128], F32, tag=pstag,
                                          name="tp")
                            nc.tensor.transpose(
                                tp, stage[:, j * 128:(j + 1) * 128], ident)
                            nc.vector.tensor_copy(v_sb[m][:, nb * 4 + j, :],
                                                  tp)

                    def g_post(ps, m, nb, tpool, pps, pstag):
                        sg_t = tpool.tile([128, 512], MMDT, tag="gst",
                                          name="gst")
                        nc.scalar.activation(out=sg_t, in_=ps, func=AF.Silu)
                        nc.gpsimd.dma_start(
                            out=sgd[m * 128:(m + 1) * 128,
                                    nb * 512:(nb + 1) * 512],
                            in_=sg_t)

                    _proj_sweep(nc, tc, xT, [wv, wg], [v_post, g_post])

                # ---- attention (per batch) + AllGather ----
                with tc.tile_pool(name="at_ssy", bufs=1, space="PSUM") as pssy, \
                     tc.tile_pool(name="at_yt", bufs=2, space="PSUM") as pyt, \
                     tc.tile_pool(name="at_ps2", bufs=2, space="PSUM") as pps2, \
                     tc.tile_pool(name="at_slab", bufs=1) as slab, \
                     tc.tile_pool(name="at_t", bufs=2) as tpool, \
                     tc.tile_pool(name="at_bh", bufs=1) as bhpool, \
                     tc.tile_pool(name="at_c", bufs=1) as acp:
                    mask_sb = acp.tile([128, 128], MMDT)
                    nc.scalar.dma_start(out=mask_sb, in_=mask01)

                    def tail_half(b, h, yst, wyb):
                        """Normalize + gate + export tokens [h*TH,(h+1)*TH)
                        of batch b for both heads, then gather them."""
                        t0 = b * T
                        hsl = slice(h * TH, (h + 1) * TH)
                        for m in range(HC):
                            sgl = bhpool.tile([128, TH], MMDT,
                                              tag=f"sgl{m}", name=f"sgl{m}")
                            nc.gpsimd.dma_start(
                                out=sgl,
                                in_=sgd[m * 128:(m + 1) * 128,
                                        t0 + h * TH:t0 + (h + 1) * TH])
                            fb2 = bhpool.tile([128, TH], MMDT,
                                              tag=f"fb2{m}", name=f"fb2{m}")
                            nc.scalar.activation(out=fb2, in_=wyb[m][:, hsl],
                                                 func=AF.Abs_reciprocal_sqrt,
                                                 scale=1.0 / float(D),
                                                 bias=zerob)
                            yf1 = bhpool.tile([128, TH], MMDT,
                                              tag=f"yf1{m}", name=f"yf1{m}")
                            nc.vector.tensor_mul(yf1, yst[m][:, hsl], fb2)
                            yf = bhpool.tile([128, TH], MMDT,
                                             tag=f"yf{m}", name=f"yf{m}")
                            nc.vector.tensor_mul(yf, yf1, sgl)
                            nc.gpsimd.dma_start(
                                out=ag_in[b][h][m * 128:(m + 1) * 128, :],
                                in_=yf)
                        nc.gpsimd.collective_compute(
                            "AllGather", ALU.bypass,
                            ins=[ag_in[b][h]], outs=[yall[b][h]],
                            replica_groups=[list(range(NCORES))],
                        )

                    for b in range(B):
                        t0 = b * T
                        yst, wyb = {}, {}
                        for m in range(HC):
                            yst[m] = bhpool.tile([128, T], MMDT,
                                                 tag=f"yst{m}",
                                                 name=f"yst{m}")
                            wyb[m] = bhpool.tile([128, T], MMDT,
                                                 tag=f"wyb{m}",
                                                 name=f"wyb{m}")

                        def emit_qtail(m, qb, ytp):
                            """Evacuate a finished (head, q-block): stash y,
                            square, row-sum.  Deferred one block so the ssy
                            matmul never head-of-line blocks fresh scores."""
                            qsl = slice(qb * 512, (qb + 1) * 512)
                            nc.vector.tensor_copy(yst[m][:, qsl], ytp)
                            sqy = tpool.tile([128, 512], MMDT, tag="ysq",
                                             name="ysq")
                            nc.vector.tensor_mul(sqy, yst[m][:, qsl],
                                                 yst[m][:, qsl])
                            ssyp = pssy.tile([128, 512], F32, tag="ssy",
                                             name="ssyp")
                            nc.tensor.matmul(ssyp, ones_r, sqy,
                                             start=True, stop=True)
                            nc.vector.tensor_copy(wyb[m][:, qsl], ssyp)

                        pend = None
                        for qb in range(T // 512):
                            nk = 4 * (qb + 1)
                            for m in range(HC):
                                ytp = pyt.tile([128, 512], F32, tag="yt",
                                               name="ytp")

                                def pv_pair(i2, e):
                                    for j in range(2):
                                        i = i2 + j
                                        nc.tensor.matmul(
                                            ytp,
                                            v_sb[m][:, b * 16 + i, :],
                                            e[:, j * 512:(j + 1) * 512],
                                            start=(i == 0),
                                            stop=(i == nk - 1))

                                es = []
                                for i2 in range(0, nk, 2):
                                    stp = pps2.tile([128, 1024], F32,
                                                    tag="st", name="stp")
                                    e = slab.tile([128, 1024], MMDT,
                                                  tag=f"es{i2 // 2}",
                                                  name=f"es{i2 // 2}")
                                    for j in range(2):
                                        i = i2 + j
                                        sl = slice(j * 512, (j + 1) * 512)
                                        nc.tensor.matmul(
                                            stp[:, sl],
                                            kTf[m][:, t0 + i * 128:
                                                   t0 + (i + 1) * 128],
                                            qTf[m][:, t0 + qb * 512:
                                                   t0 + (qb + 1) * 512],
                                            start=True, stop=True)
                                    q_off0 = i2 * 128 - qb * 512
                                    lo = max(0, q_off0)
                                    nc.scalar.activation(
                                        out=e[:, lo:], in_=stp[:, lo:],
                                        func=AF.Exp, scale=SCALE)
                                    # causal fixups on the e slab (gpsimd,
                                    # off the PE->ACT critical path)
                                    for j in range(2):
                                        i = i2 + j
                                        q_off = i * 128 - qb * 512
                                        if q_off >= 0:
                                            if j == 0 and q_off > 0:
                                                nc.gpsimd.memset(
                                                    e[:, 0:q_off], 0.0)
                                            if j == 1 and q_off > 0:
                                                nc.gpsimd.memset(
                                                    e[:, 512:512 + q_off],
                                                    0.0)
                                            dsl = slice(
                                                j * 512 + q_off,
                                                j * 512 + q_off + 128)
                                            nc.gpsimd.tensor_mul(
                                                e[:, dsl], e[:, dsl],
                                                mask_sb)
                                    es.append((i2, e))
                                    # software-pipeline the P@V matmuls one
                                    # pair behind the score/exp production
                                    if len(es) >= 2:
                                        pv_pair(*es[-2])
                                pv_pair(*es[-1])

                                if pend is not None:
                                    emit_qtail(*pend)
                                    if pend[1] == 1 and pend[0] == HC - 1:
                                        # first token-half done (both heads):
                                        # gather while second half computes
                                        tail_half(b, 0, yst, wyb)
                                pend = (m, qb, ytp)
                        emit_qtail(*pend)
                        tail_half(b, 1, yst, wyb)

                # ---- final projection: Wo column slice, per token-half ----
                with tc.tile_pool(name="fin_ps", bufs=2, space="PSUM") as fps, \
                     tc.tile_pool(name="fin_y", bufs=3) as ypool, \
                     tc.tile_pool(name="fin_o", bufs=2) as opool:
                    for b in range(B):
                        for h in range(2):
                            ya = yall[b][h].rearrange("(kt p) t -> p kt t",
                                                      p=128)
                            po = [fps.tile([128, TH], F32, tag=f"po{m}",
                                           name=f"po{m}") for m in range(HC)]
                            for kd in range(KT):
                                ysl = ypool.tile([128, TH], MMDT, tag="ysl",
                                                 name="ysl", bufs=4)
                                nc.gpsimd.dma_start(out=ysl, in_=ya[:, kd, :])
                                for m in range(HC):
                                    lhsT = wo_sb[:, kd, m * 128:(m + 1) * 128]
                                    for tb in range(TH // 512):
                                        nc.tensor.matmul(
                                            po[m][:, tb * 512:(tb + 1) * 512],
                                            lhsT,
                                            ysl[:, tb * 512:(tb + 1) * 512],
                                            start=(kd == 0),
                                            stop=(kd == KT - 1))
                            for m in range(HC):
                                ot = opool.tile([128, TH], F32, tag="ot",
                                                name="ot")
                                nc.vector.tensor_copy(ot, po[m])
                                nc.scalar.dma_start(
                                    out=outT[m * 128:(m + 1) * 128,
                                             b * T + h * TH:
                                             b * T + (h + 1) * TH],
                                    in_=ot)
    nc.compile()
    return nc


def _get_nc():
    global _CACHED_NC
    if _CACHED_NC is None:
        _CACHED_NC = _build_nc()
    return _CACHED_NC


def kernel(x, Wq, Wk, Wv, Wg, Wo, q_rms_w, k_rms_w, o_norm_w):
    global LAST_EXEC_TIME_NS, LAST_RESULT
    import ml_dtypes
    npdt = ml_dtypes.bfloat16
    x = np.asarray(x, dtype=np.float32)
    Wq = np.asarray(Wq, dtype=np.float32)
    Wk = np.asarray(Wk, dtype=np.float32)
    Wv = np.asarray(Wv, dtype=np.float32)
    Wg = np.asarray(Wg, dtype=np.float32)
    Wo = np.asarray(Wo, dtype=np.float32)
    q_rms_w = np.asarray(q_rms_w, dtype=np.float32)
    k_rms_w = np.asarray(k_rms_w, dtype=np.float32)
    o_norm_w = np.asarray(o_norm_w, dtype=np.float32)

    xT = x.reshape(BT, HID).T          # [HID, BT]
    # [KT, BT//1024, 128, 1024] contiguous chunks
    xt4 = np.ascontiguousarray(
        xT.reshape(KT, 128, BT // 1024, 1024).transpose(0, 2, 1, 3)).astype(npdt)
    # fold o_norm_w into Wo rows: (y*o_w) @ Wo == y @ (o_w[:,None]*Wo)
    wo_scaled = Wo * np.tile(o_norm_w, H)[:, None]

    inv = 1.0 / (10000.0 ** (np.arange(0, D, 2, dtype=np.float64) / D))
    pos = np.arange(T, dtype=np.float64)
    fr = pos[:, None] * inv[None, :]          # [T, 64]
    cosT = np.cos(fr).T.astype(np.float32)    # [64, T]
    sinT = np.sin(fr).T.astype(np.float32)
    cosbt = np.concatenate([cosT] * B, axis=1)
    sinbt = np.concatenate([sinT] * B, axis=1)
    cos2 = np.ascontiguousarray(np.vstack([cosbt, cosbt]))   # [128, BT]
    # sign-folded sin: rows 0-63 carry -sin (for t1*c - t2*s), rows 64-127 +sin
    sin2 = np.ascontiguousarray(np.vstack([-sinbt, sinbt]))

    kk, qq = np.meshgrid(np.arange(128), np.arange(128), indexing="ij")
    mask01 = (kk <= qq).astype(np.float32)
    ones128 = np.ones((128, 128), dtype=np.float32)
    ident = np.eye(128, dtype=np.float32)

    in_maps = []
    for c in range(NCORES):
        csl = slice(c * DC, (c + 1) * DC)
        def wt(wmat):
            # [HID, DC] -> [128, KT, DC] matching the SBUF tile layout
            return np.ascontiguousarray(
                wmat[:, csl].reshape(KT, 128, DC).transpose(1, 0, 2)).astype(npdt)
        in_maps.append({
            "xT": xt4,
            "wq": wt(Wq),
            "wk": wt(Wk),
            "wv": wt(Wv),
            "wg": wt(Wg),
            "wo": wt(wo_scaled),
            "cos2": cos2,
            "sin2": sin2,
            "mask01": mask01.astype(npdt),
            "ones_in": ones128.astype(npdt),
            "ident_in": ident,
            "qrw": np.ascontiguousarray(q_rms_w.reshape(128, 1)),
            "krw": np.ascontiguousarray(k_rms_w.reshape(128, 1)),
        })

    nc = _get_nc()
    trace = os.environ.get("KERNEL_TRACE", "0") == "1"
    res = run_bass_kernel_spmd(nc, in_maps, list(range(NCORES)), trace=trace)
    LAST_EXEC_TIME_NS = res.exec_time_ns
    LAST_RESULT = res

    outT_full = np.concatenate([res.results[c]["outT"] for c in range(NCORES)],
                               axis=0)              # [2048 n, 4096 t]
    out = outT_full.T.reshape(B, T, HID)
    return np.ascontiguousarray(out)


# revision 28
# speedup vs baseline: 1.0528x; 1.0528x over previous
"""Trainium2 Bass kernel for a dense transformer block (QKV+gate proj, RoPE,
QK-RMSNorm, causal SDPA, output-RMSNorm + SiLU gate, output projection).

Sharding: tensor-parallel over heads across 8 NeuronCores (2 heads/core).
Wq/Wk/Wv/Wg split column-wise, attention fully local per core; the per-core
attention outputs are exchanged with an AllToAll (token-sharding), after
which each core computes the FULL output projection for its 256-token slice
(per batch) with the full Wo resident in SBUF.  This replaces the 8x-larger
AllGather wire volume of the row/column-parallel variants.

Matmuls run in bf16.  RoPE uses a sign-folded sin table (rows 0-63 hold
-sin, 64-127 hold +sin) so the rotate-half combine is a single add.
The softmax denominator is never computed: RMSNorm(y/s) == RMSNorm(y) up to
the (negligible) eps term, so only sum(y^2) is reduced via a ones-matmul.
"""

import os
import sys

for _p in ("/opt/trn_rl_repo", "/root/.axon_site/_ro/trn_rl_repo"):
    if os.path.isdir(_p) and _p not in sys.path:
        sys.path.insert(0, _p)

import numpy as np

import concourse.bass as bass
import concourse.mybir as mybir
from concourse import bacc
from concourse.bass_utils import run_bass_kernel_spmd
from concourse.tile import TileContext

B, T, HID = 2, 2048, 2048
H, D = 16, 128
NCORES = 8
HC = H // NCORES          # heads per core = 2
DC = HC * D               # 256 head-dims per core
TC = T // NCORES          # 256 tokens per core (final projection shard)
BT = B * T                # 4096 tokens
KT = HID // 128           # 16 contraction tiles
EPS = 1e-5
SCALE = 1.0 / float(np.sqrt(D))
NEG = -3.0e38

F32 = mybir.dt.float32
BF16 = mybir.dt.bfloat16
AF = mybir.ActivationFunctionType
ALU = mybir.AluOpType

MMDT = BF16

LAST_EXEC_TIME_NS = None
LAST_RESULT = None
_CACHED_NC = None


def _proj_sweeps(nc, tc, xT_r, segments):
    """Continuous stream over xT: each segment computes 2 matrices (4
    head-groups) in transposed layout, psum[dhead 128, tok 512]
    double-buffered, accumulated over 16 k-tiles.  Segments share the pools
    so there is no pipeline drain between them."""
    with tc.tile_pool(name="sweep_w", bufs=1) as wpool, \
         tc.tile_pool(name="sweep_x", bufs=3) as xpool, \
         tc.tile_pool(name="sweep_ps", bufs=2, space="PSUM") as pps, \
         tc.tile_pool(name="sweep_t", bufs=2) as tpool:
        wsb = {}
        for seg, (w_aps, _) in enumerate(segments):
            for mi, w_ap in enumerate(w_aps):
                w_t = wpool.tile([128, KT, DC], MMDT, tag=f"w{seg}{mi}",
                                 name=f"w{seg}{mi}")
                # first segment's weights gate the first matmuls: sync queue;
                # later segments prefetch on the gpsimd queue
                if seg == 0:
                    nc.sync.dma_start(out=w_t, in_=w_ap)
                else:
                    nc.gpsimd.dma_start(out=w_t, in_=w_ap)
                wsb[(seg, mi)] = w_t
        pending = None
        for seg, (w_aps, post) in enumerate(segments):
            for nb in range(BT // 512):
                nbb, half = nb // 2, nb % 2
                ps = {}
                for mi in range(2):
                    for m in range(HC):
                        ps[(mi, m)] = pps.tile([128, 512], F32,
                                               tag=f"pp{mi}{m}",
                                               name=f"pp{mi}{m}")
                for k in range(KT):
                    xk = xpool.tile([128, 512], MMDT, tag=f"xk{half}",
                                    name="xk", bufs=6)
                    nc.sync.dma_start(
                        out=xk,
                        in_=xT_r[k, nbb, :, half * 512:(half + 1) * 512])
                    for mi in range(2):
                        for m in range(HC):
                            nc.tensor.matmul(
                                ps[(mi, m)],
                                wsb[(seg, mi)][:, k, m * 128:(m + 1) * 128],
                                xk, start=(k == 0), stop=(k == KT - 1))
                # defer evacuation by one token-block: keeps the posts' PE
                # work (row-sum matmuls / transposes) from head-of-line
                # blocking the next block's main matmuls while ACT catches up
                if pending is not None:
                    postp, nbp, psp = pending
                    for mi in range(2):
                        for m in range(HC):
                            postp[mi](psp[(mi, m)], m, nbp, tpool, pps,
                                      f"pp{mi}{m}")
                pending = (post, nb, ps)
        postp, nbp, psp = pending
        for mi in range(2):
            for m in range(HC):
                postp[mi](psp[(mi, m)], m, nbp, tpool, pps, f"pp{mi}{m}")


def _build_nc():
    nc = bacc.Bacc("TRN2", target_bir_lowering=False, debug=False,
                   num_devices=NCORES)

    xT = nc.dram_tensor("xT", [KT, BT // 1024, 128, 1024], MMDT,
                        kind="ExternalInput").ap()
    wq = nc.dram_tensor("wq", [128, KT, DC], MMDT, kind="ExternalInput").ap()
    wk = nc.dram_tensor("wk", [128, KT, DC], MMDT, kind="ExternalInput").ap()
    wv = nc.dram_tensor("wv", [128, KT, DC], MMDT, kind="ExternalInput").ap()
    wg = nc.dram_tensor("wg", [128, KT, DC], MMDT, kind="ExternalInput").ap()
    # per-core column slice of the output projection (o_norm folded in)
    wo = nc.dram_tensor("wo", [128, KT, DC], MMDT, kind="ExternalInput").ap()
    cos2 = nc.dram_tensor("cos2", [128, BT], F32, kind="ExternalInput").ap()
    # sign-folded: rows 0-63 = -sin, rows 64-127 = +sin
    sin2 = nc.dram_tensor("sin2", [128, BT], F32, kind="ExternalInput").ap()
    # causal 0/1 mask for diagonal tiles (k<=q -> 1)
    mask01 = nc.dram_tensor("mask01", [128, 128], MMDT, kind="ExternalInput").ap()
    ones_in = nc.dram_tensor("ones_in", [128, 128], MMDT, kind="ExternalInput").ap()
    ident_in = nc.dram_tensor("ident_in", [128, 128], F32, kind="ExternalInput").ap()
    qrw = nc.dram_tensor("qrw", [128, 1], F32, kind="ExternalInput").ap()
    krw = nc.dram_tensor("krw", [128, 1], F32, kind="ExternalInput").ap()

    outT = nc.dram_tensor("outT", [DC, BT], F32, kind="ExternalOutput").ap()
    sgd = nc.dram_tensor("sgd", [DC, BT], MMDT).ap()       # silu(gate) spill
    TH = T // 2          # AllGather granularity: half a batch of tokens
    ag_in = [[nc.dram_tensor(f"ag_in{b}_{h}", [DC, TH], MMDT).ap()
              for h in range(2)] for b in range(B)]
    yall = [[nc.dram_tensor(f"yall{b}_{h}", [NCORES * DC, TH], MMDT,
                            addr_space="Shared").ap() for h in range(2)]
            for b in range(B)]

    with TileContext(nc) as tc:
        with tc.tile_pool(name="const", bufs=1) as const:
            ones_r = const.tile([128, 128], MMDT)
            nc.sync.dma_start(out=ones_r, in_=ones_in)
            epsb = const.tile([128, 1], F32)
            nc.vector.memset(epsb, EPS)
            zerob = const.tile([128, 1], F32)
            nc.vector.memset(zerob, 0.0)

            with tc.tile_pool(name="persist", bufs=1) as persist:
                # Wo column slice in SBUF; prefetch on the gpsimd DMA queue
                wo_sb = persist.tile([128, KT, DC], MMDT, tag="wo", name="wo")
                nc.gpsimd.dma_start(out=wo_sb, in_=wo)

                # final (rope+rms applied) qT/kT per head, bf16 [d, b*t]
                qTf = [persist.tile([128, BT], MMDT, tag=f"qTf{m}",
                                    name=f"qTf{m}") for m in range(HC)]
                kTf = [persist.tile([128, BT], MMDT, tag=f"kTf{m}",
                                    name=f"kTf{m}") for m in range(HC)]

                # ---- sweep A: q, k (rope + rms fused into evacuation) ----
                if True:
                    # persist (not a scoped pool) so closing doesn't stall
                    # the sweep-B pools on the rope evacuation drain.
                    # off the sync queue so w/x tile loads start immediately
                    cos_sb = persist.tile([128, BT], F32, tag="cos")
                    nc.scalar.dma_start(out=cos_sb, in_=cos2)
                    sin_sb = persist.tile([128, BT], F32, tag="sin")
                    nc.scalar.dma_start(out=sin_sb, in_=sin2)
                    qrw_sb = persist.tile([128, 1], F32, tag="qrw")
                    nc.scalar.dma_start(out=qrw_sb, in_=qrw)
                    krw_sb = persist.tile([128, 1], F32, tag="krw")
                    nc.scalar.dma_start(out=krw_sb, in_=krw)

                    def make_qk_post(dest, w_scalar):
                        def post(ps, m, nb, tpool, pps, pstag):
                            c0, c1 = nb * 512, (nb + 1) * 512
                            stage = tpool.tile([128, 512], F32, tag="stage",
                                               name="stage")
                            nc.scalar.copy(stage, ps)
                            sw = tpool.tile([128, 512], F32, tag="sw",
                                            name="sw")
                            nc.scalar.dma_start(out=sw[0:64, :],
                                                in_=stage[64:128, :])
                            nc.scalar.dma_start(out=sw[64:128, :],
                                                in_=stage[0:64, :])
                            sqq = tpool.tile([128, 512], MMDT, tag="sq",
                                             name="sq")
                            nc.scalar.activation(out=sqq, in_=ps,
                                                 func=AF.Square)
                            ss = pps.tile([128, 512], F32, tag=pstag,
                                          name="ss")
                            nc.tensor.matmul(ss, ones_r, sqq,
                                             start=True, stop=True)
                            fac = tpool.tile([128, 512], F32, tag="fac",
                                             name="fac")
                            nc.scalar.activation(out=fac, in_=ss,
                                                 func=AF.Abs_reciprocal_sqrt,
                                                 scale=1.0 / float(D),
                                                 bias=epsb)
                            u = tpool.tile([128, 512], F32, tag="u", name="u")
                            nc.vector.tensor_mul(u, stage, cos_sb[:, c0:c1])
                            w = tpool.tile([128, 512], F32, tag="w", name="w")
                            nc.gpsimd.tensor_mul(w, sw, sin_sb[:, c0:c1])
                            ro = tpool.tile([128, 512], F32, tag="ro",
                                            name="ro")
                            nc.vector.tensor_add(ro, u, w)
                            # dest = (ro * w[d]) * factor  (fused)
                            nc.vector.scalar_tensor_tensor(
                                out=dest[m][:, c0:c1], in0=ro, scalar=w_scalar,
                                in1=fac, op0=ALU.mult, op1=ALU.mult)
                        return post

                # ---- sweep seg B: v (transpose to [tok, d]), silu(g) ----
                v_sb = [persist.tile([128, BT // 128, 128], MMDT, tag=f"v{m}",
                                     name=f"v{m}") for m in range(HC)]
                ident = persist.tile([128, 128], F32, tag="ident")
                nc.scalar.dma_start(out=ident, in_=ident_in)

                def v_post(ps, m, nb, tpool, pps, pstag):
                    stage = tpool.tile([128, 512], F32, tag="vstage",
                                       name="stage")
                    nc.scalar.copy(stage, ps)
                    for j in range(4):
                        tp = pps.tile([128, 128], F32, tag=pstag,
                                      name="tp")
                        nc.tensor.transpose(
                            tp, stage[:, j * 128:(j + 1) * 128], ident)
                        nc.vector.tensor_copy(v_sb[m][:, nb * 4 + j, :],
                                              tp)

                def g_post(ps, m, nb, tpool, pps, pstag):
                    sg_t = tpool.tile([128, 512], MMDT, tag="gst",
                                      name="gst")
                    nc.scalar.activation(out=sg_t, in_=ps, func=AF.Silu)
                    nc.gpsimd.dma_start(
                        out=sgd[m * 128:(m + 1) * 128,
                                nb * 512:(nb + 1) * 512],
                        in_=sg_t)

                _proj_sweeps(nc, tc, xT, [
                    ([wq, wk], [make_qk_post(qTf, qrw_sb),
                                make_qk_post(kTf, krw_sb)]),
                    ([wv, wg], [v_post, g_post]),
                ])

                # ---- attention (per batch) + AllGather ----
                with tc.tile_pool(name="at_ssy", bufs=1, space="PSUM") as pssy, \
                     tc.tile_pool(name="at_yt", bufs=3, space="PSUM") as pyt, \
                     tc.tile_pool(name="at_ps2", bufs=2, space="PSUM") as pps2, \
                     tc.tile_pool(name="at_slab", bufs=1) as slab, \
                     tc.tile_pool(name="at_t", bufs=2) as tpool, \
                     tc.tile_pool(name="at_bh", bufs=1) as bhpool, \
                     tc.tile_pool(name="at_c", bufs=1) as acp:
                    mask_sb = acp.tile([128, 128], MMDT)
                    nc.scalar.dma_start(out=mask_sb, in_=mask01)

                    def tail_half(b, h, yst, wyb):
                        """Normalize + gate + export tokens [h*TH,(h+1)*TH)
                        of batch b for both heads, then gather them."""
                        t0 = b * T
                        hsl = slice(h * TH, (h + 1) * TH)
                        for m in range(HC):
                            sgl = bhpool.tile([128, TH], MMDT,
                                              tag=f"sgl{m}", name=f"sgl{m}")
                            nc.gpsimd.dma_start(
                                out=sgl,
                                in_=sgd[m * 128:(m + 1) * 128,
                                        t0 + h * TH:t0 + (h + 1) * TH])
                            fb2 = bhpool.tile([128, TH], MMDT,
                                              tag=f"fb2{m}", name=f"fb2{m}")
                            nc.scalar.activation(out=fb2, in_=wyb[m][:, hsl],
                                                 func=AF.Abs_reciprocal_sqrt,
                                                 scale=1.0 / float(D),
                                                 bias=zerob)
                            yf1 = bhpool.tile([128, TH], MMDT,
                                              tag=f"yf1{m}", name=f"yf1{m}")
                            nc.vector.tensor_mul(yf1, yst[m][:, hsl], fb2)
                            yf = bhpool.tile([128, TH], MMDT,
                                             tag=f"yf{m}", name=f"yf{m}")
                            nc.vector.tensor_mul(yf, yf1, sgl)
                            nc.gpsimd.dma_start(
                                out=ag_in[b][h][m * 128:(m + 1) * 128, :],
                                in_=yf)
                        nc.gpsimd.collective_compute(
                            "AllGather", ALU.bypass,
                            ins=[ag_in[b][h]], outs=[yall[b][h]],
                            replica_groups=[list(range(NCORES))],
                        )

                    for b in range(B):
                        t0 = b * T
                        yst, wyb = {}, {}
                        for m in range(HC):
                            yst[m] = bhpool.tile([128, T], MMDT,
                                                 tag=f"yst{m}",
                                                 name=f"yst{m}")
                            wyb[m] = bhpool.tile([128, T], MMDT,
                                                 tag=f"wyb{m}",
                                                 name=f"wyb{m}")

                        def emit_qtail(m, qb, ytp):
                            """Evacuate a finished (head, q-block): stash y,
                            square, row-sum.  Deferred one block so the ssy
                            matmul never head-of-line blocks fresh scores."""
                            qsl = slice(qb * 512, (qb + 1) * 512)
                            nc.vector.tensor_copy(yst[m][:, qsl], ytp)
                            sqy = tpool.tile([128, 512], MMDT, tag="ysq",
                                             name="ysq")
                            nc.vector.tensor_mul(sqy, yst[m][:, qsl],
                                                 yst[m][:, qsl])
                            ssyp = pssy.tile([128, 512], F32, tag="ssy",
                                             name="ssyp")
                            nc.tensor.matmul(ssyp, ones_r, sqy,
                                             start=True, stop=True)
                            nc.vector.tensor_copy(wyb[m][:, qsl], ssyp)

                        def pv_pair(ctx, i2, e):
                            qb, m, nk, ytp = ctx
                            for j in range(2):
                                i = i2 + j
                                nc.tensor.matmul(
                                    ytp,
                                    v_sb[m][:, b * 16 + i, :],
                                    e[:, j * 512:(j + 1) * 512],
                                    start=(i == 0),
                                    stop=(i == nk - 1))

                        # flat pipeline over every (q-block, head, k-pair):
                        # the P@V consumer runs DEPTH pairs behind the
                        # score/exp producer, crossing block boundaries so
                        # the PE never drains at a (head, q-block) edge
                        DEPTH = 2
                        stream = []
                        pend_tail = None
                        gpair = 0
                        def drain_one():
                            nonlocal pend_tail
                            ctx, i2, e = stream.pop(0)
                            pv_pair(ctx, i2, e)
                            if i2 == ctx[2] - 2:   # block's last pair
                                if pend_tail is not None:
                                    pm, pqb, pytp = pend_tail
                                    emit_qtail(pm, pqb, pytp)
                                    if pqb == 1 and pm == HC - 1:
                                        # first token-half done (both
                                        # heads): gather it now
                                        tail_half(b, 0, yst, wyb)
                                pend_tail = (ctx[1], ctx[0], ctx[3])
                        for qb in range(T // 512):
                            nk = 4 * (qb + 1)
                            for m in range(HC):
                                ytp = pyt.tile([128, 512], F32, tag="yt",
                                               name="ytp")
                                ctx = (qb, m, nk, ytp)
                                for i2 in range(0, nk, 2):
                                    stp = pps2.tile([128, 1024], F32,
                                                    tag="st", name="stp")
                                    e = slab.tile([128, 1024], MMDT,
                                                  tag=f"es{gpair % 8}",
                                                  name=f"es{gpair % 8}")
                                    gpair += 1
                                    for j in range(2):
                                        i = i2 + j
                                        sl = slice(j * 512, (j + 1) * 512)
                                        nc.tensor.matmul(
                                            stp[:, sl],
                                            kTf[m][:, t0 + i * 128:
                                                   t0 + (i + 1) * 128],
                                            qTf[m][:, t0 + qb * 512:
                                                   t0 + (qb + 1) * 512],
                                            start=True, stop=True)
                                    q_off0 = i2 * 128 - qb * 512
                                    lo = max(0, q_off0)
                                    nc.scalar.activation(
                                        out=e[:, lo:], in_=stp[:, lo:],
                                        func=AF.Exp, scale=SCALE)
                                    # causal fixups on the e slab (gpsimd,
                                    # off the PE->ACT critical path)
                                    for j in range(2):
                                        i = i2 + j
                                        q_off = i * 128 - qb * 512
                                        if q_off >= 0:
                                            if j == 0 and q_off > 0:
                                                nc.gpsimd.memset(
                                                    e[:, 0:q_off], 0.0)
                                            if j == 1 and q_off > 0:
                                                nc.gpsimd.memset(
                                                    e[:, 512:512 + q_off],
                                                    0.0)
                                            dsl = slice(
                                                j * 512 + q_off,
                                                j * 512 + q_off + 128)
                                            nc.gpsimd.tensor_mul(
                                                e[:, dsl], e[:, dsl],
                                                mask_sb)
                                    stream.append((ctx, i2, e))
                                    if len(stream) > DEPTH:
                                        drain_one()
                        while stream:
                            drain_one()
                        pm, pqb, pytp = pend_tail
                        emit_qtail(pm, pqb, pytp)
                        tail_half(b, 1, yst, wyb)
                        pend_tail = None

                # ---- final projection: Wo column slice, per token-half ----
                with tc.tile_pool(name="fin_ps", bufs=2, space="PSUM") as fps, \
                     tc.tile_pool(name="fin_y", bufs=3) as ypool, \
                     tc.tile_pool(name="fin_o", bufs=2) as opool:
                    for b in range(B):
                        for h in range(2):
                            ya = yall[b][h].rearrange("(kt p) t -> p kt t",
                                                      p=128)
                            po = [fps.tile([128, TH], F32, tag=f"po{m}",
                                           name=f"po{m}") for m in range(HC)]
                            for kd in range(KT):
                                ysl = ypool.tile([128, TH], MMDT, tag="ysl",
                                                 name="ysl", bufs=4)
                                nc.gpsimd.dma_start(out=ysl, in_=ya[:, kd, :])
                                for m in range(HC):
                                    lhsT = wo_sb[:, kd, m * 128:(m + 1) * 128]
                                    for tb in range(TH // 512):
                                        nc.tensor.matmul(
                                            po[m][:, tb * 512:(tb + 1) * 512],
                                            lhsT,
                                            ysl[:, tb * 512:(tb + 1) * 512],
                                            start=(kd == 0),
                                            stop=(kd == KT - 1))
                            for m in range(HC):
                                ot = opool.tile([128, TH], F32, tag="ot",
                                                name="ot")
                                nc.vector.tensor_copy(ot, po[m])
                                nc.scalar.dma_start(
                                    out=outT[m * 128:(m + 1) * 128,
                                             b * T + h * TH:
                                             b * T + (h + 1) * TH],
                                    in_=ot)
    nc.compile()
    return nc


def _get_nc():
    global _CACHED_NC
    if _CACHED_NC is None:
        _CACHED_NC = _build_nc()
    return _CACHED_NC


def kernel(x, Wq, Wk, Wv, Wg, Wo, q_rms_w, k_rms_w, o_norm_w):
    global LAST_EXEC_TIME_NS, LAST_RESULT
    import ml_dtypes
    npdt = ml_dtypes.bfloat16
    x = np.asarray(x, dtype=np.float32)
    Wq = np.asarray(Wq, dtype=np.float32)
    Wk = np.asarray(Wk, dtype=np.float32)
    Wv = np.asarray(Wv, dtype=np.float32)
    Wg = np.asarray(Wg, dtype=np.float32)
    Wo = np.asarray(Wo, dtype=np.float32)
    q_rms_w = np.asarray(q_rms_w, dtype=np.float32)
    k_rms_w = np.asarray(k_rms_w, dtype=np.float32)
    o_norm_w = np.asarray(o_norm_w, dtype=np.float32)

    xT = x.reshape(BT, HID).T          # [HID, BT]
    # [KT, BT//1024, 128, 1024] contiguous chunks
    xt4 = np.ascontiguousarray(
        xT.reshape(KT, 128, BT // 1024, 1024).transpose(0, 2, 1, 3)).astype(npdt)
    # fold o_norm_w into Wo rows: (y*o_w) @ Wo == y @ (o_w[:,None]*Wo)
    wo_scaled = Wo * np.tile(o_norm_w, H)[:, None]

    inv = 1.0 / (10000.0 ** (np.arange(0, D, 2, dtype=np.float64) / D))
    pos = np.arange(T, dtype=np.float64)
    fr = pos[:, None] * inv[None, :]          # [T, 64]
    cosT = np.cos(fr).T.astype(np.float32)    # [64, T]
    sinT = np.sin(fr).T.astype(np.float32)
    cosbt = np.concatenate([cosT] * B, axis=1)
    sinbt = np.concatenate([sinT] * B, axis=1)
    cos2 = np.ascontiguousarray(np.vstack([cosbt, cosbt]))   # [128, BT]
    # sign-folded sin: rows 0-63 carry -sin (for t1*c - t2*s), rows 64-127 +sin
    sin2 = np.ascontiguousarray(np.vstack([-sinbt, sinbt]))

    kk, qq = np.meshgrid(np.arange(128), np.arange(128), indexing="ij")
    mask01 = (kk <= qq).astype(np.float32)
    ones128 = np.ones((128, 128), dtype=np.float32)
    ident = np.eye(128, dtype=np.float32)

    in_maps = []
    for c in range(NCORES):
        csl = slice(c * DC, (c + 1) * DC)
        def wt(wmat):
            # [HID, DC] -> [128, KT, DC] matching the SBUF tile layout
            return np.ascontiguousarray(
                wmat[:, csl].reshape(KT, 128, DC).transpose(1, 0, 2)).astype(npdt)
        in_maps.append({
            "xT": xt4,
            "wq": wt(Wq),
            "wk": wt(Wk),
            "wv": wt(Wv),
            "wg": wt(Wg),
            "wo": wt(wo_scaled),
            "cos2": cos2,
            "sin2": sin2,
            "mask01": mask01.astype(npdt),
            "ones_in": ones128.astype(npdt),
            "ident_in": ident,
            "qrw": np.ascontiguousarray(q_rms_w.reshape(128, 1)),
            "krw": np.ascontiguousarray(k_rms_w.reshape(128, 1)),
        })

    nc = _get_nc()
    trace = os.environ.get("KERNEL_TRACE", "0") == "1"
    res = run_bass_kernel_spmd(nc, in_maps, list(range(NCORES)), trace=trace)
    LAST_EXEC_TIME_NS = res.exec_time_ns
    LAST_RESULT = res

    outT_full = np.concatenate([res.results[c]["outT"] for c in range(NCORES)],
                               axis=0)              # [2048 n, 4096 t]
    out = outT_full.T.reshape(B, T, HID)
    return np.ascontiguousarray(out)


# revision 33
# speedup vs baseline: 1.0783x; 1.0242x over previous
"""Trainium2 Bass kernel for a dense transformer block (QKV+gate proj, RoPE,
QK-RMSNorm, causal SDPA, output-RMSNorm + SiLU gate, output projection).

Sharding: tensor-parallel over heads across 8 NeuronCores (2 heads/core).
Wq/Wk/Wv/Wg split column-wise, attention fully local per core; the per-core
attention outputs are exchanged with an AllToAll (token-sharding), after
which each core computes the FULL output projection for its 256-token slice
(per batch) with the full Wo resident in SBUF.  This replaces the 8x-larger
AllGather wire volume of the row/column-parallel variants.

Matmuls run in bf16.  RoPE uses a sign-folded sin table (rows 0-63 hold
-sin, 64-127 hold +sin) so the rotate-half combine is a single add.
The softmax denominator is never computed: RMSNorm(y/s) == RMSNorm(y) up to
the (negligible) eps term, so only sum(y^2) is reduced via a ones-matmul.
"""

import os
import sys

for _p in ("/opt/trn_rl_repo", "/root/.axon_site/_ro/trn_rl_repo"):
    if os.path.isdir(_p) and _p not in sys.path:
        sys.path.insert(0, _p)

import numpy as np

import concourse.bass as bass
import concourse.mybir as mybir
from concourse import bacc
from concourse.bass_utils import run_bass_kernel_spmd
from concourse.tile import TileContext

B, T, HID = 2, 2048, 2048
H, D = 16, 128
NCORES = 8
HC = H // NCORES          # heads per core = 2
DC = HC * D               # 256 head-dims per core
TC = T // NCORES          # 256 tokens per core (final projection shard)
BT = B * T                # 4096 tokens
KT = HID // 128           # 16 contraction tiles
EPS = 1e-5
SCALE = 1.0 / float(np.sqrt(D))
NEG = -3.0e38

F32 = mybir.dt.float32
BF16 = mybir.dt.bfloat16
AF = mybir.ActivationFunctionType
ALU = mybir.AluOpType

MMDT = BF16

LAST_EXEC_TIME_NS = None
LAST_RESULT = None
_CACHED_NC = None


def _proj_sweeps(nc, tc, xT_r, segments):
    """Continuous stream over xT: each segment computes 2 matrices (4
    head-groups) in transposed layout, psum[dhead 128, tok 512]
    double-buffered, accumulated over 16 k-tiles.  Segments share the pools
    so there is no pipeline drain between them."""
    with tc.tile_pool(name="sweep_w", bufs=1) as wpool, \
         tc.tile_pool(name="sweep_x", bufs=3) as xpool, \
         tc.tile_pool(name="sweep_ps", bufs=2, space="PSUM") as pps, \
         tc.tile_pool(name="sweep_t", bufs=2) as tpool:
        wsb = {}
        for seg, (w_aps, _) in enumerate(segments):
            for mi, w_ap in enumerate(w_aps):
                w_t = wpool.tile([128, KT, DC], MMDT, tag=f"w{seg}{mi}",
                                 name=f"w{seg}{mi}")
                # first segment's weights gate the first matmuls: sync queue;
                # later segments prefetch on the gpsimd queue
                if seg == 0:
                    nc.sync.dma_start(out=w_t, in_=w_ap)
                else:
                    nc.gpsimd.dma_start(out=w_t, in_=w_ap)
                wsb[(seg, mi)] = w_t
        pending = None
        for seg, (w_aps, post) in enumerate(segments):
            for nb in range(BT // 512):
                nbb, half = nb // 2, nb % 2
                ps = {}
                for mi in range(2):
                    for m in range(HC):
                        ps[(mi, m)] = pps.tile([128, 512], F32,
                                               tag=f"pp{mi}{m}",
                                               name=f"pp{mi}{m}")
                for k in range(KT):
                    xk = xpool.tile([128, 512], MMDT, tag=f"xk{half}",
                                    name="xk", bufs=6)
                    nc.sync.dma_start(
                        out=xk,
                        in_=xT_r[k, nbb, :, half * 512:(half + 1) * 512])
                    for mi in range(2):
                        for m in range(HC):
                            nc.tensor.matmul(
                                ps[(mi, m)],
                                wsb[(seg, mi)][:, k, m * 128:(m + 1) * 128],
                                xk, start=(k == 0), stop=(k == KT - 1))
                # defer evacuation by one token-block: keeps the posts' PE
                # work (row-sum matmuls / transposes) from head-of-line
                # blocking the next block's main matmuls while ACT catches up
                if pending is not None:
                    postp, nbp, psp = pending
                    for mi in range(2):
                        for m in range(HC):
                            postp[mi](psp[(mi, m)], m, nbp, tpool, pps,
                                      f"pp{mi}{m}")
                pending = (post, nb, ps)
        postp, nbp, psp = pending
        for mi in range(2):
            for m in range(HC):
                postp[mi](psp[(mi, m)], m, nbp, tpool, pps, f"pp{mi}{m}")


def _build_nc():
    nc = bacc.Bacc("TRN2", target_bir_lowering=False, debug=False,
                   num_devices=NCORES)

    xT = nc.dram_tensor("xT", [KT, BT // 1024, 128, 1024], MMDT,
                        kind="ExternalInput").ap()
    wq = nc.dram_tensor("wq", [128, KT, DC], MMDT, kind="ExternalInput").ap()
    wk = nc.dram_tensor("wk", [128, KT, DC], MMDT, kind="ExternalInput").ap()
    wv = nc.dram_tensor("wv", [128, KT, DC], MMDT, kind="ExternalInput").ap()
    wg = nc.dram_tensor("wg", [128, KT, DC], MMDT, kind="ExternalInput").ap()
    # per-core column slice of the output projection (o_norm folded in)
    wo = nc.dram_tensor("wo", [128, KT, DC], MMDT, kind="ExternalInput").ap()
    cos2 = nc.dram_tensor("cos2", [128, BT], F32, kind="ExternalInput").ap()
    # sign-folded: rows 0-63 = -sin, rows 64-127 = +sin
    sin2 = nc.dram_tensor("sin2", [128, BT], F32, kind="ExternalInput").ap()
    # causal 0/1 mask for diagonal tiles (k<=q -> 1)
    mask01 = nc.dram_tensor("mask01", [128, 128], MMDT, kind="ExternalInput").ap()
    ones_in = nc.dram_tensor("ones_in", [128, 128], MMDT, kind="ExternalInput").ap()
    ident_in = nc.dram_tensor("ident_in", [128, 128], F32, kind="ExternalInput").ap()
    qrw = nc.dram_tensor("qrw", [128, 1], F32, kind="ExternalInput").ap()
    krw = nc.dram_tensor("krw", [128, 1], F32, kind="ExternalInput").ap()

    outT = nc.dram_tensor("outT", [DC, BT], F32, kind="ExternalOutput").ap()
    sgd = nc.dram_tensor("sgd", [DC, BT], MMDT).ap()       # silu(gate) spill
    TH = T // 2          # AllGather granularity: half a batch of tokens
    ag_in = [[nc.dram_tensor(f"ag_in{b}_{h}", [DC, TH], MMDT).ap()
              for h in range(2)] for b in range(B)]
    yall = [[nc.dram_tensor(f"yall{b}_{h}", [NCORES * DC, TH], MMDT,
                            addr_space="Shared").ap() for h in range(2)]
            for b in range(B)]

    with TileContext(nc) as tc:
        with tc.tile_pool(name="const", bufs=1) as const:
            ones_r = const.tile([128, 128], MMDT)
            nc.sync.dma_start(out=ones_r, in_=ones_in)
            epsb = const.tile([128, 1], F32)
            nc.vector.memset(epsb, EPS)
            zerob = const.tile([128, 1], F32)
            nc.vector.memset(zerob, 0.0)

            with tc.tile_pool(name="persist", bufs=1) as persist:
                # Wo column slice in SBUF; loaded later (gpsimd queue), it
                # is only needed by the final projection
                wo_sb = persist.tile([128, KT, DC], MMDT, tag="wo", name="wo")

                # final (rope+rms applied) qT/kT per head, bf16 [d, b*t]
                qTf = [persist.tile([128, BT], MMDT, tag=f"qTf{m}",
                                    name=f"qTf{m}") for m in range(HC)]
                kTf = [persist.tile([128, BT], MMDT, tag=f"kTf{m}",
                                    name=f"kTf{m}") for m in range(HC)]

                # ---- sweep A: q, k (rope + rms fused into evacuation) ----
                if True:
                    # persist (not a scoped pool) so closing doesn't stall
                    # the sweep-B pools on the rope evacuation drain.
                    # off the sync queue so w/x tile loads start immediately
                    cos_sb = persist.tile([128, BT], F32, tag="cos")
                    nc.gpsimd.dma_start(out=cos_sb, in_=cos2)
                    sin_sb = persist.tile([128, BT], F32, tag="sin")
                    nc.gpsimd.dma_start(out=sin_sb, in_=sin2)
                    qrw_sb = persist.tile([128, 1], F32, tag="qrw")
                    nc.scalar.dma_start(out=qrw_sb, in_=qrw)
                    krw_sb = persist.tile([128, 1], F32, tag="krw")
                    nc.scalar.dma_start(out=krw_sb, in_=krw)

                    def make_qk_post(dest, w_scalar):
                        def post(ps, m, nb, tpool, pps, pstag):
                            c0, c1 = nb * 512, (nb + 1) * 512
                            stage = tpool.tile([128, 512], F32, tag="stage",
                                               name="stage")
                            nc.scalar.copy(stage, ps)
                            sw = tpool.tile([128, 512], F32, tag="sw",
                                            name="sw")
                            nc.scalar.dma_start(out=sw[0:64, :],
                                                in_=stage[64:128, :])
                            nc.scalar.dma_start(out=sw[64:128, :],
                                                in_=stage[0:64, :])
                            sqq = tpool.tile([128, 512], MMDT, tag="sq",
                                             name="sq")
                            nc.scalar.activation(out=sqq, in_=ps,
                                                 func=AF.Square)
                            ss = pps.tile([128, 512], F32, tag=pstag,
                                          name="ss")
                            nc.tensor.matmul(ss, ones_r, sqq,
                                             start=True, stop=True)
                            fac = tpool.tile([128, 512], F32, tag="fac",
                                             name="fac")
                            nc.scalar.activation(out=fac, in_=ss,
                                                 func=AF.Abs_reciprocal_sqrt,
                                                 scale=1.0 / float(D),
                                                 bias=epsb)
                            u = tpool.tile([128, 512], F32, tag="u", name="u")
                            nc.vector.tensor_mul(u, stage, cos_sb[:, c0:c1])
                            w = tpool.tile([128, 512], F32, tag="w", name="w")
                            nc.gpsimd.tensor_mul(w, sw, sin_sb[:, c0:c1])
                            ro = tpool.tile([128, 512], F32, tag="ro",
                                            name="ro")
                            nc.vector.tensor_add(ro, u, w)
                            # dest = (ro * w[d]) * factor  (fused)
                            nc.vector.scalar_tensor_tensor(
                                out=dest[m][:, c0:c1], in0=ro, scalar=w_scalar,
                                in1=fac, op0=ALU.mult, op1=ALU.mult)
                        return post

                # ---- sweep seg B: v (transpose to [tok, d]), silu(g) ----
                v_sb = [persist.tile([128, BT // 128, 128], MMDT, tag=f"v{m}",
                                     name=f"v{m}") for m in range(HC)]
                ident = persist.tile([128, 128], F32, tag="ident")
                nc.scalar.dma_start(out=ident, in_=ident_in)

                def v_post(ps, m, nb, tpool, pps, pstag):
                    stage = tpool.tile([128, 512], F32, tag="vstage",
                                       name="stage")
                    nc.scalar.copy(stage, ps)
                    for j in range(4):
                        tp = pps.tile([128, 128], F32, tag=pstag,
                                      name="tp")
                        nc.tensor.transpose(
                            tp, stage[:, j * 128:(j + 1) * 128], ident)
                        nc.vector.tensor_copy(v_sb[m][:, nb * 4 + j, :],
                                              tp)

                def g_post(ps, m, nb, tpool, pps, pstag):
                    sg_t = tpool.tile([128, 512], MMDT, tag="gst",
                                      name="gst")
                    nc.scalar.activation(out=sg_t, in_=ps, func=AF.Silu)
                    nc.gpsimd.dma_start(
                        out=sgd[m * 128:(m + 1) * 128,
                                nb * 512:(nb + 1) * 512],
                        in_=sg_t)

                _proj_sweeps(nc, tc, xT, [
                    ([wq, wk], [make_qk_post(qTf, qrw_sb),
                                make_qk_post(kTf, krw_sb)]),
                    ([wv, wg], [v_post, g_post]),
                ])
                nc.gpsimd.dma_start(out=wo_sb, in_=wo)

                # ---- attention (per batch) + AllGather ----
                with tc.tile_pool(name="at_ssy", bufs=1, space="PSUM") as pssy, \
                     tc.tile_pool(name="at_yt", bufs=3, space="PSUM") as pyt, \
                     tc.tile_pool(name="at_ps2", bufs=2, space="PSUM") as pps2, \
                     tc.tile_pool(name="at_slab", bufs=1) as slab, \
                     tc.tile_pool(name="at_t", bufs=2) as tpool, \
                     tc.tile_pool(name="at_bh", bufs=1) as bhpool, \
                     tc.tile_pool(name="at_c", bufs=1) as acp:
                    mask_sb = acp.tile([128, 128], MMDT)
                    nc.scalar.dma_start(out=mask_sb, in_=mask01)

                    def tail_half(b, h, yst, wyb):
                        """Normalize + gate + export tokens [h*TH,(h+1)*TH)
                        of batch b for both heads, then gather them.  Both
                        rsqrt calls are emitted back-to-back so they share
                        one ACT table-set switch."""
                        t0 = b * T
                        hsl = slice(h * TH, (h + 1) * TH)
                        sgl, fb2 = {}, {}
                        for m in range(HC):
                            sgl[m] = bhpool.tile([128, TH], MMDT,
                                                 tag=f"sgl{m}",
                                                 name=f"sgl{m}")
                            nc.gpsimd.dma_start(
                                out=sgl[m],
                                in_=sgd[m * 128:(m + 1) * 128,
                                        t0 + h * TH:t0 + (h + 1) * TH])
                        for m in range(HC):
                            fb2[m] = bhpool.tile([128, TH], MMDT,
                                                 tag=f"fb2{m}",
                                                 name=f"fb2{m}")
                            nc.scalar.activation(out=fb2[m],
                                                 in_=wyb[m][:, hsl],
                                                 func=AF.Abs_reciprocal_sqrt,
                                                 scale=1.0 / float(D),
                                                 bias=zerob)
                        for m in range(HC):
                            yf1 = bhpool.tile([128, TH], MMDT,
                                              tag=f"yf1{m}", name=f"yf1{m}")
                            nc.vector.tensor_mul(yf1, yst[m][:, hsl], fb2[m])
                            yf = bhpool.tile([128, TH], MMDT,
                                             tag=f"yf{m}", name=f"yf{m}")
                            nc.vector.tensor_mul(yf, yf1, sgl[m])
                            nc.gpsimd.dma_start(
                                out=ag_in[b][h][m * 128:(m + 1) * 128, :],
                                in_=yf)
                        nc.gpsimd.collective_compute(
                            "AllGather", ALU.bypass,
                            ins=[ag_in[b][h]], outs=[yall[b][h]],
                            replica_groups=[list(range(NCORES))],
                        )

                    for b in range(B):
                        t0 = b * T
                        yst, wyb = {}, {}
                        for m in range(HC):
                            yst[m] = bhpool.tile([128, T], MMDT,
                                                 tag=f"yst{m}",
                                                 name=f"yst{m}")
                            wyb[m] = bhpool.tile([128, T], MMDT,
                                                 tag=f"wyb{m}",
                                                 name=f"wyb{m}")

                        def emit_qtail(m, qb, ytp):
                            """Evacuate a finished (head, q-block): stash y,
                            square, row-sum.  Deferred one block so the ssy
                            matmul never head-of-line blocks fresh scores."""
                            qsl = slice(qb * 512, (qb + 1) * 512)
                            nc.vector.tensor_copy(yst[m][:, qsl], ytp)
                            sqy = tpool.tile([128, 512], MMDT, tag="ysq",
                                             name="ysq")
                            nc.vector.tensor_mul(sqy, yst[m][:, qsl],
                                                 yst[m][:, qsl])
                            ssyp = pssy.tile([128, 512], F32, tag="ssy",
                                             name="ssyp")
                            nc.tensor.matmul(ssyp, ones_r, sqy,
                                             start=True, stop=True)
                            nc.vector.tensor_copy(wyb[m][:, qsl], ssyp)

                        def pv_pair(ctx, i2, e):
                            qb, m, nk, ytp = ctx
                            for j in range(2):
                                i = i2 + j
                                nc.tensor.matmul(
                                    ytp,
                                    v_sb[m][:, b * 16 + i, :],
                                    e[:, j * 512:(j + 1) * 512],
                                    start=(i == 0),
                                    stop=(i == nk - 1))

                        # flat pipeline over every (q-block, head, k-pair):
                        # the P@V consumer runs DEPTH pairs behind the
                        # score/exp producer, crossing block boundaries so
                        # the PE never drains at a (head, q-block) edge
                        DEPTH = 2
                        stream = []
                        pend_tail = None
                        gpair = 0
                        def drain_one():
                            nonlocal pend_tail
                            ctx, i2, e = stream.pop(0)
                            pv_pair(ctx, i2, e)
                            if i2 == ctx[2] - 2:   # block's last pair
                                if pend_tail is not None:
                                    pm, pqb, pytp = pend_tail
                                    emit_qtail(pm, pqb, pytp)
                                    if pqb == 1 and pm == HC - 1:
                                        # first token-half done (both
                                        # heads): gather it now
                                        tail_half(b, 0, yst, wyb)
                                pend_tail = (ctx[1], ctx[0], ctx[3])
                        for qb in range(T // 512):
                            nk = 4 * (qb + 1)
                            for m in range(HC):
                                ytp = pyt.tile([128, 512], F32, tag="yt",
                                               name="ytp")
                                ctx = (qb, m, nk, ytp)
                                for i2 in range(0, nk, 2):
                                    stp = pps2.tile([128, 1024], F32,
                                                    tag="st", name="stp")
                                    e = slab.tile([128, 1024], MMDT,
                                                  tag=f"es{gpair % 8}",
                                                  name=f"es{gpair % 8}")
                                    gpair += 1
                                    for j in range(2):
                                        i = i2 + j
                                        sl = slice(j * 512, (j + 1) * 512)
                                        nc.tensor.matmul(
                                            stp[:, sl],
                                            kTf[m][:, t0 + i * 128:
                                                   t0 + (i + 1) * 128],
                                            qTf[m][:, t0 + qb * 512:
                                                   t0 + (qb + 1) * 512],
                                            start=True, stop=True)
                                    q_off0 = i2 * 128 - qb * 512
                                    lo = max(0, q_off0)
                                    nc.scalar.activation(
                                        out=e[:, lo:], in_=stp[:, lo:],
                                        func=AF.Exp, scale=SCALE)
                                    # causal fixups on the e slab (gpsimd,
                                    # off the PE->ACT critical path)
                                    for j in range(2):
                                        i = i2 + j
                                        q_off = i * 128 - qb * 512
                                        if q_off >= 0:
                                            if j == 0 and q_off > 0:
                                                nc.gpsimd.memset(
                                                    e[:, 0:q_off], 0.0)
                                            if j == 1 and q_off > 0:
                                                nc.gpsimd.memset(
                                                    e[:, 512:512 + q_off],
                                                    0.0)
                                            dsl = slice(
                                                j * 512 + q_off,
                                                j * 512 + q_off + 128)
                                            nc.gpsimd.tensor_mul(
                                                e[:, dsl], e[:, dsl],
                                                mask_sb)
                                    stream.append((ctx, i2, e))
                                    if len(stream) > DEPTH:
                                        drain_one()
                        while stream:
                            drain_one()
                        pm, pqb, pytp = pend_tail
                        emit_qtail(pm, pqb, pytp)
                        tail_half(b, 1, yst, wyb)
                        pend_tail = None

                # ---- final projection: Wo column slice, per token-half ----
                with tc.tile_pool(name="fin_ps", bufs=2, space="PSUM") as fps, \
                     tc.tile_pool(name="fin_y", bufs=3) as ypool, \
                     tc.tile_pool(name="fin_o", bufs=2) as opool:
                    for b in range(B):
                        for h in range(2):
                            ya = yall[b][h].rearrange("(kt p) t -> p kt t",
                                                      p=128)
                            po = [fps.tile([128, TH], F32, tag=f"po{m}",
                                           name=f"po{m}") for m in range(HC)]
                            for kd in range(KT):
                                ysl = ypool.tile([128, TH], MMDT, tag="ysl",
                                                 name="ysl", bufs=6)
                                nc.gpsimd.dma_start(out=ysl, in_=ya[:, kd, :])
                                for m in range(HC):
                                    lhsT = wo_sb[:, kd, m * 128:(m + 1) * 128]
                                    for tb in range(TH // 512):
                                        nc.tensor.matmul(
                                            po[m][:, tb * 512:(tb + 1) * 512],
                                            lhsT,
                                            ysl[:, tb * 512:(tb + 1) * 512],
                                            start=(kd == 0),
                                            stop=(kd == KT - 1))
                            for m in range(HC):
                                ot = opool.tile([128, TH], F32, tag="ot",
                                                name="ot")
                                nc.vector.tensor_copy(ot, po[m])
                                nc.scalar.dma_start(
                                    out=outT[m * 128:(m + 1) * 128,
                                             b * T + h * TH:
                                             b * T + (h + 1) * TH],
                                    in_=ot)
    nc.compile()
    return nc


def _get_nc():
    global _CACHED_NC
    if _CACHED_NC is None:
        _CACHED_NC = _build_nc()
    return _CACHED_NC


def kernel(x, Wq, Wk, Wv, Wg, Wo, q_rms_w, k_rms_w, o_norm_w):
    global LAST_EXEC_TIME_NS, LAST_RESULT
    import ml_dtypes
    npdt = ml_dtypes.bfloat16
    x = np.asarray(x, dtype=np.float32)
    Wq = np.asarray(Wq, dtype=np.float32)
    Wk = np.asarray(Wk, dtype=np.float32)
    Wv = np.asarray(Wv, dtype=np.float32)
    Wg = np.asarray(Wg, dtype=np.float32)
    Wo = np.asarray(Wo, dtype=np.float32)
    q_rms_w = np.asarray(q_rms_w, dtype=np.float32)
    k_rms_w = np.asarray(k_rms_w, dtype=np.float32)
    o_norm_w = np.asarray(o_norm_w, dtype=np.float32)

    xT = x.reshape(BT, HID).T          # [HID, BT]
    # [KT, BT//1024, 128, 1024] contiguous chunks
    xt4 = np.ascontiguousarray(
        xT.reshape(KT, 128, BT // 1024, 1024).transpose(0, 2, 1, 3)).astype(npdt)
    # fold o_norm_w into Wo rows: (y*o_w) @ Wo == y @ (o_w[:,None]*Wo)
    wo_scaled = Wo * np.tile(o_norm_w, H)[:, None]

    inv = 1.0 / (10000.0 ** (np.arange(0, D, 2, dtype=np.float64) / D))
    pos = np.arange(T, dtype=np.float64)
    fr = pos[:, None] * inv[None, :]          # [T, 64]
    cosT = np.cos(fr).T.astype(np.float32)    # [64, T]
    sinT = np.sin(fr).T.astype(np.float32)
    cosbt = np.concatenate([cosT] * B, axis=1)
    sinbt = np.concatenate([sinT] * B, axis=1)
    cos2 = np.ascontiguousarray(np.vstack([cosbt, cosbt]))   # [128, BT]
    # sign-folded sin: rows 0-63 carry -sin (for t1*c - t2*s), rows 64-127 +sin
    sin2 = np.ascontiguousarray(np.vstack([-sinbt, sinbt]))

    kk, qq = np.meshgrid(np.arange(128), np.arange(128), indexing="ij")
    mask01 = (kk <= qq).astype(np.float32)
    ones128 = np.ones((128, 128), dtype=np.float32)
    ident = np.eye(128, dtype=np.float32)

    in_maps = []
    for c in range(NCORES):
        csl = slice(c * DC, (c + 1) * DC)
        def wt(wmat):
            # [HID, DC] -> [128, KT, DC] matching the SBUF tile layout
            return np.ascontiguousarray(
                wmat[:, csl].reshape(KT, 128, DC).transpose(1, 0, 2)).astype(npdt)
        in_maps.append({
            "xT": xt4,
            "wq": wt(Wq),
            "wk": wt(Wk),
            "wv": wt(Wv),
            "wg": wt(Wg),
            "wo": wt(wo_scaled),
            "cos2": cos2,
            "sin2": sin2,
            "mask01": mask01.astype(npdt),
            "ones_in": ones128.astype(npdt),
            "ident_in": ident,
            "qrw": np.ascontiguousarray(q_rms_w.reshape(128, 1)),
            "krw": np.ascontiguousarray(k_rms_w.reshape(128, 1)),
        })

    nc = _get_nc()
    trace = os.environ.get("KERNEL_TRACE", "0") == "1"
    res = run_bass_kernel_spmd(nc, in_maps, list(range(NCORES)), trace=trace)
    LAST_EXEC_TIME_NS = res.exec_time_ns
    LAST_RESULT = res

    outT_full = np.concatenate([res.results[c]["outT"] for c in range(NCORES)],
                               axis=0)              # [2048 n, 4096 t]
    out = outT_full.T.reshape(B, T, HID)
    return np.ascontiguousarray(out)
